# revision 1
# baseline (speedup 1.0000x reference)
"""Trainium2 Bass kernel for AdjacencyMatchingLoss (8-core SPMD).

Math: adj_score[b,e] = P[b,i_e,:] @ A @ P[b,j_e,:]  with A = (d_hw==1).
Let W[i,j] = sum_e w_e * 1[i_e=i] * 1[j_e=j]   (weighted pair histogram)
Then  total_adj = sum_ij W[i,j] * mean_b (P_b A P_b^T)[i,j]
               = (1/B) * sum_b < P_b^T W P_b , A >

Per core: edges are sharded E/8 ways (the P activations and d_hw are
replicated). The pair histogram W is built on the TensorEngine by
accumulating 49 one-hot outer-product matmuls (K = 128 edges each).
The one-hot operands are built on the VectorEngine as 7-chunk slab
tensor_tensor ops against a broadcast iota (stride-0 access patterns),
which HW-measured ~3x faster than per-chunk tensor_scalar ops — the
per-instruction overhead dominates at [128,128] granularity.
C = sum_b P_b^T W P_b follows with 10 more matmuls; thanks to the U =
W^T P_b factorization all operands arrive in natural layout and no
transposes are needed anywhere. The kernel emits [128,2] per-partition
partials of [<C,-A/8>, sum(w)]; the host sums partials over partitions
and cores and performs the final divide (that reduction is part of
unsharding the scalar output).

Inputs are packed host-side into one [128,768] int16 "meta" tensor per
core (pairs as int16 words of the int64s | w | d_hw rows), read on
device through bitcast views, so the critical path needs one small DMA.

Alternative structures kept behind flags (all HW-measured slower):
SPLIT_W (overlap half of U/C with the one-hot phase), BIGOP=False
(per-chunk tensor_scalar one-hots spread across DVE/gpsimd/ACT, with an
ACT one-hot trick: relu(w - w*|i - idx|) = w*onehot since w >= 0).
"""

import os
import sys

import numpy as np

for _p in ("/opt/trn_rl_repo",):
    if os.path.isdir(_p) and _p not in sys.path:
        sys.path.insert(0, _p)

B, NL, NQ, E = 8, 128, 128, 50000
NCORES = 8
ESH = E // NCORES            # 6250 edges per core
CHUNKS = (ESH + 127) // 128  # 49
EPAD = CHUNKS * 128          # 6272
SPLIT_W = False
BIGOP = True  # build one-hots as slab tensor_tensor ops on DVE
BIGOP_SLABS = (7, 7, 7, 7, 7, 7, 7)
DIAG = None  # None | 'nohot' (static one-hots) | 'nomm' (single W matmul)
META_W = 768                 # i16 words/partition: 392 pairs | 98 w | 256 d | pad

_BUILT = None


def _emit_body(nc, sp, pp, tensors):
    import concourse.mybir as mybir

    f32 = mybir.dt.float32
    bf16 = mybir.dt.bfloat16
    i32 = mybir.dt.int32
    i16 = mybir.dt.int16
    EQ = mybir.AluOpType.is_equal
    MUL = mybir.AluOpType.mult
    ADD = mybir.AluOpType.add
    ABS = mybir.ActivationFunctionType.Abs
    RELU = mybir.ActivationFunctionType.Relu
    P_d, meta_d, o_d = tensors

    Pf = sp.tile([128, B * NQ], f32)
    Pb = sp.tile([128, B * NQ], bf16)
    meta = sp.tile([128, META_W], i16)
    Asc = sp.tile([128, NQ], f32)
    idx = sp.tile([128, 2 * CHUNKS], f32)   # interleaved [c][i,j]
    if not BIGOP:
        wNeg = sp.tile([128, CHUNKS], f32)
    iot = sp.tile([128, 128], bf16)
    OhJ = sp.tile([128, EPAD], bf16)
    OhIW = sp.tile([128, EPAD], bf16)
    WsbA = sp.tile([128, 128], bf16)
    UsbA = sp.tile([128, B * NQ], bf16)
    if SPLIT_W:
        WsbB = sp.tile([128, 128], bf16)
        UsbB = sp.tile([128, B * NQ], bf16)
    prt = sp.tile([128, 2], f32)
    scr = sp.tile([128, NQ], f32)

    WpsA = pp.tile([128, 128], f32)
    if SPLIT_W:
        WpsB = pp.tile([128, 128], f32)
    else:
        WpsB = WpsA
    Up0 = pp.tile([128, 512], f32)
    Up1 = pp.tile([128, 512], f32)
    Cps = pp.tile([128, 128], f32)

    # ---- loads ----
    # pairs+w words first (they gate the one-hot phase); the d_hw words
    # ride in the same packed tensor but are only needed at the tail.
    nc.sync.dma_start(out=meta[:, 0:490], in_=meta_d.ap()[:, 0:490])
    P_src = P_d.ap().rearrange("b l q -> l b q")
    Pf3 = Pf[:].rearrange("l (b q) -> l b q", q=NQ)
    nc.sync.dma_start(out=Pf3[:, 0:4, :], in_=P_src[:, 0:4, :])
    nc.sync.dma_start(out=Pf3[:, 4:8, :], in_=P_src[:, 4:8, :])
    nc.sync.dma_start(out=meta[:, 490:746], in_=meta_d.ap()[:, 490:746])

    # views into the packed meta row
    prs3 = meta[:, 0:392].rearrange("p (c k) -> p c k", k=8)
    wT = meta[:, 392:490].bitcast(f32)          # [128, 49] f32
    dsb = meta[:, 490:746].bitcast(i32)         # [128, 128] i32

    # ---- prep ----
    nc.gpsimd.iota(
        iot[:],
        pattern=[[1, 128]],
        base=0,
        channel_multiplier=0,
        allow_small_or_imprecise_dtypes=True,
    )
    # both index columns (i at word 0, j at word 4) in one strided copy
    nc.vector.tensor_copy(
        out=idx[:].rearrange("p (c k) -> p c k", k=2),
        in_=meta[:, 0:392].rearrange("p (c k u) -> p c k u", k=2, u=4)[:, :, :, 0:1],
    )
    if not BIGOP:
        nc.gpsimd.tensor_scalar(
            out=wNeg[:], in0=wT, scalar1=-1.0, scalar2=None, op0=MUL
        )
    nc.vector.tensor_reduce(
        out=prt[:, 1:2], in_=wT, axis=mybir.AxisListType.X, op=ADD
    )

    def idxI(c):
        return idx[:, 2 * c : 2 * c + 1]

    def idxJ(c):
        return idx[:, 2 * c + 1 : 2 * c + 2]

    # ---- one-hots + W accumulation (two halves) + overlapped U/C ----
    if SPLIT_W:
        ACT_CHUNKS = {5, 15, 25, 35}
        POOL_CHUNKS = {3, 8, 13, 18, 23, 28, 33, 38, 43}
        HALF = 25
    else:
        ACT_CHUNKS = {5, 15, 25, 35}
        POOL_CHUNKS = {3, 8, 13, 18, 23, 28, 33, 38, 43}
        HALF = CHUNKS

    def emit_chunk(c):
        sl = slice(c * 128, (c + 1) * 128)
        if DIAG == "nohot" and c > 0:
            sl0 = slice(0, 128)
            Wp = WpsA if c < HALF else WpsB
            nc.tensor.matmul(
                Wp[:], lhsT=OhIW[:, sl0], rhs=OhJ[:, sl0],
                start=(c in (0, HALF)), stop=(c in (HALF - 1, CHUNKS - 1)),
            )
            return
        if DIAG == "nomm" and c > 0:
            if c in ACT_CHUNKS:
                pass
            eng0 = nc.gpsimd if c in POOL_CHUNKS else nc.vector
            if c in ACT_CHUNKS:
                tmpJx = sp.tile([128, 128], bf16, name=f"tmpJ{c}")
                tmpIx = sp.tile([128, 128], bf16, name=f"tmpI{c}")
                nc.scalar.activation(out=tmpJx[:], in_=iot[:], func=ABS, bias=idxJ(c), scale=-1.0)
                nc.scalar.activation(out=OhJ[:, sl], in_=tmpJx[:], func=RELU, bias=1.0, scale=-1.0)
                nc.scalar.activation(out=tmpIx[:], in_=iot[:], func=ABS, bias=idxI(c), scale=-1.0)
                nc.scalar.activation(out=OhIW[:, sl], in_=tmpIx[:], func=RELU, bias=wT[:, c:c+1], scale=wNeg[:, c:c+1])
            else:
                eng0.tensor_scalar(out=OhJ[:, sl], in0=iot[:], scalar1=idxJ(c), scalar2=None, op0=EQ)
                eng0.tensor_scalar(out=OhIW[:, sl], in0=iot[:], scalar1=idxI(c), scalar2=wT[:, c:c+1], op0=EQ, op1=MUL)
            return
        if c in ACT_CHUNKS:
            tmpJ = sp.tile([128, 128], bf16, name=f"tmpJ{c}")
            tmpI = sp.tile([128, 128], bf16, name=f"tmpI{c}")
            nc.scalar.activation(
                out=tmpJ[:], in_=iot[:], func=ABS, bias=idxJ(c), scale=-1.0
            )
            nc.scalar.activation(
                out=OhJ[:, sl], in_=tmpJ[:], func=RELU, bias=1.0, scale=-1.0
            )
            nc.scalar.activation(
                out=tmpI[:], in_=iot[:], func=ABS, bias=idxI(c), scale=-1.0
            )
            nc.scalar.activation(
                out=OhIW[:, sl], in_=tmpI[:], func=RELU,
                bias=wT[:, c : c + 1], scale=wNeg[:, c : c + 1],
            )
        else:
            eng = nc.gpsimd if c in POOL_CHUNKS else nc.vector
            eng.tensor_scalar(
                out=OhJ[:, sl], in0=iot[:], scalar1=idxJ(c), scalar2=None, op0=EQ
            )
            eng.tensor_scalar(
                out=OhIW[:, sl], in0=iot[:], scalar1=idxI(c),
                scalar2=wT[:, c : c + 1], op0=EQ, op1=MUL,
            )
        Wp = WpsA if c < HALF else WpsB
        nc.tensor.matmul(
            Wp[:],
            lhsT=OhIW[:, sl],
            rhs=OhJ[:, sl],
            start=(c in (0, HALF)),
            stop=(c in (HALF - 1, CHUNKS - 1)),
        )

    def emit_uc(Wsb, Wps, Usb, first, last):
        nc.vector.tensor_copy(out=Wsb[:], in_=Wps[:])
        nc.tensor.matmul(
            Up0[:], lhsT=Wsb[:], rhs=Pb[:, 0:512], start=True, stop=True
        )
        nc.tensor.matmul(
            Up1[:], lhsT=Wsb[:], rhs=Pb[:, 512:1024], start=True, stop=True
        )
        if not last:
            # phase still running: keep DVE free, ACT absorbs both copies
            nc.scalar.copy(out=Usb[:, 0:512], in_=Up0[:])
            nc.scalar.copy(out=Usb[:, 512:1024], in_=Up1[:])
        else:
            nc.vector.tensor_copy(out=Usb[:, 0:512], in_=Up0[:])
            nc.scalar.copy(out=Usb[:, 512:1024], in_=Up1[:])
        for b in range(B):
            sl = slice(b * 128, (b + 1) * 128)
            nc.tensor.matmul(
                Cps[:],
                lhsT=Usb[:, sl],
                rhs=Pb[:, sl],
                start=(first and b == 0),
                stop=(last and b == B - 1),
            )

    if BIGOP:
        import concourse.bass as bass

        # bf16 copies of the indices/weights so slab ops run all-bf16
        idxb = sp.tile([128, 2 * CHUNKS], bf16)
        wTb = sp.tile([128, CHUNKS], bf16)
        OhI = sp.tile([128, EPAD], bf16)
        nc.vector.tensor_copy(out=idxb[:], in_=idx[:])
        nc.vector.tensor_copy(out=wTb[:], in_=wT)

        def bcast(t_ap, step, n_c, rep):
            # [128,1]-based AP -> [128, n_c (stride step), rep (stride 0)]
            return bass.AP(
                t_ap.tensor, t_ap.offset, [t_ap.ap[0], [step, n_c], [0, rep]]
            )

        SLABS = BIGOP_SLABS
        assert sum(SLABS) == CHUNKS
        # With SPLIT_W: slabs [0, split_slab) accumulate into WpsA and the
        # A-half U/C chain runs on the otherwise-idle ACT + PE while DVE is
        # still building the remaining slabs' one-hots.
        split_slab = 4 if SPLIT_W else len(SLABS)
        chalf = sum(SLABS[:split_slab])

        def emit_uc_overlap(Wsb, Wps, Usb):
            # everything off DVE: ACT does all three PSUM->SBUF bf16 copies
            nc.scalar.copy(out=Wsb[:], in_=Wps[:])
            nc.tensor.matmul(
                Up0[:], lhsT=Wsb[:], rhs=Pb[:, 0:512], start=True, stop=True
            )
            nc.tensor.matmul(
                Up1[:], lhsT=Wsb[:], rhs=Pb[:, 512:1024], start=True, stop=True
            )
            nc.scalar.copy(out=Usb[:, 0:512], in_=Up0[:])
            nc.scalar.copy(out=Usb[:, 512:1024], in_=Up1[:])
            for b in range(B):
                slb = slice(b * 128, (b + 1) * 128)
                nc.tensor.matmul(
                    Cps[:], lhsT=Usb[:, slb], rhs=Pb[:, slb],
                    start=(b == 0), stop=False,
                )

        c0 = 0
        for s, slab in enumerate(SLABS):
            sl = slice(c0 * 128, (c0 + slab) * 128)
            oh3 = lambda t: t[:, sl].rearrange("p (c i) -> p c i", i=128)
            iot_rep = bass.AP(
                iot[:].tensor, iot[:].offset, [iot[:].ap[0], [0, slab], [1, 128]]
            )
            nc.vector.tensor_tensor(
                out=oh3(OhJ), in0=iot_rep,
                in1=bcast(idxb[:, 2 * c0 + 1 : 2 * c0 + 2], 2, slab, 128), op=EQ,
            )
            nc.vector.tensor_tensor(
                out=oh3(OhI), in0=iot_rep,
                in1=bcast(idxb[:, 2 * c0 : 2 * c0 + 1], 2, slab, 128), op=EQ,
            )
            nc.vector.tensor_tensor(
                out=oh3(OhIW), in0=oh3(OhI),
                in1=bcast(wTb[:, c0 : c0 + 1], 1, slab, 128), op=MUL,
            )
            if s == (1 if SPLIT_W else len(SLABS) - 2):
                # Pb conversion on the idle gpsimd, in time for U matmuls
                nc.gpsimd.tensor_copy(out=Pb[:, 0:512], in_=Pf[:, 0:512])
                nc.gpsimd.tensor_copy(out=Pb[:, 512:1024], in_=Pf[:, 512:1024])
            for c in range(c0, c0 + slab):
                slc = slice(c * 128, (c + 1) * 128)
                Wp = WpsA if c < chalf else WpsB
                nc.tensor.matmul(
                    Wp[:],
                    lhsT=OhIW[:, slc],
                    rhs=OhJ[:, slc],
                    start=(c in (0, chalf)),
                    stop=(c in (chalf - 1, CHUNKS - 1)),
                )
            c0 += slab
            if SPLIT_W and s == split_slab - 1:
                emit_uc_overlap(WsbA, WpsA, UsbA)
        if SPLIT_W:
            emit_uc(WsbB, WpsB, UsbB, first=False, last=True)
        else:
            emit_uc(WsbA, WpsA, UsbA, first=True, last=True)
    else:
        pb_at = (18, 20) if SPLIT_W else (40, 43)
        for c in range(CHUNKS):
            if c == pb_at[0]:
                # P f32->bf16 slipped into the one-hot stream, in time for
                # the first U matmuls
                nc.vector.tensor_copy(out=Pb[:, 0:512], in_=Pf[:, 0:512])
            if c == pb_at[1]:
                nc.gpsimd.tensor_copy(out=Pb[:, 512:1024], in_=Pf[:, 512:1024])
            emit_chunk(c)
            if SPLIT_W and c == HALF - 1:
                emit_uc(WsbA, WpsA, UsbA, first=True, last=False)
        if SPLIT_W:
            emit_uc(WsbB, WpsB, UsbB, first=False, last=True)
        else:
            emit_uc(WsbA, WpsA, UsbA, first=True, last=True)

    # A_scaled = -(1/8) * (d_hw == 1); folds sign + batch-mean
    nc.gpsimd.tensor_scalar(
        out=Asc[:], in0=dsb, scalar1=1, scalar2=-0.125, op0=EQ, op1=MUL
    )

    # ---- partials: [ <C, -A/8> , sum(w) ] ----
    nc.vector.tensor_tensor(out=scr[:], in0=Cps[:], in1=Asc[:], op=MUL)
    nc.vector.tensor_reduce(
        out=prt[:, 0:1], in_=scr[:], axis=mybir.AxisListType.X, op=ADD
    )
    # partition + cross-core reduction of the [128,2] partials on host
    nc.sync.dma_start(out=o_d.ap(), in_=prt[:])


def _build(reps=1):
    import concourse.bacc as bacc
    import concourse.mybir as mybir
    import concourse.tile as tile

    f32 = mybir.dt.float32
    i16 = mybir.dt.int16

    nc = bacc.Bacc("TRN2", target_bir_lowering=False, debug=False, num_devices=NCORES)

    P_d = nc.dram_tensor("p_in", [B, NL, NQ], f32, kind="ExternalInput")
    meta_d = nc.dram_tensor("meta_in", [128, META_W], i16, kind="ExternalInput")
    o_d = nc.dram_tensor("out", [128, 2], f32, kind="ExternalOutput")

    with tile.TileContext(nc) as tc:
        with (
            tc.tile_pool(name="sbuf", bufs=1) as sp,
            tc.tile_pool(name="psum", bufs=1, space="PSUM") as pp,
        ):
            for _ in range(reps):
                _emit_body(nc, sp, pp, (P_d, meta_d, o_d))

    nc.compile()
    return nc


def _get_built():
    global _BUILT
    if _BUILT is None:
        _BUILT = _build()
    return _BUILT


def _shard_inputs(P, d_hw, circuit_edge_pairs, circuit_edge_weights):
    P = np.ascontiguousarray(np.asarray(P, dtype=np.float32))
    d_hw = np.ascontiguousarray(np.asarray(d_hw, dtype=np.int32))
    pairs = np.asarray(circuit_edge_pairs).astype(np.int64, copy=False)
    w = np.asarray(circuit_edge_weights, dtype=np.float32)

    pairs_pad = np.zeros((NCORES, EPAD, 2), dtype=np.int64)
    w_pad = np.zeros((NCORES, EPAD), dtype=np.float32)
    pairs_pad[:, :ESH] = pairs.reshape(NCORES, ESH, 2)
    w_pad[:, :ESH] = w.reshape(NCORES, ESH)

    # packed per-partition row: 392 i16 of pairs | 98 i16 (49 f32 w) |
    # 256 i16 (128 i32 d row) | pad to 768
    meta = np.zeros((NCORES, 128, META_W), dtype=np.int16)
    meta[:, :, 0:392] = pairs_pad.view(np.int16).reshape(NCORES, 128, 392)
    meta[:, :, 392:490] = w_pad.view(np.int16).reshape(NCORES, 128, 98)
    meta[:, :, 490:746] = d_hw.view(np.int16).reshape(128, 256)[None]

    return [
        {"p_in": P, "meta_in": np.ascontiguousarray(meta[i])}
        for i in range(NCORES)
    ]


def _combine(results):
    parts = np.stack([np.asarray(results[i]["out"]) for i in range(NCORES)])
    numer = float(parts[:, :, 0].astype(np.float64).sum())
    wsum = float(parts[:, :, 1].astype(np.float64).sum())
    return np.float32(numer / max(wsum, 1e-8))


def make_runner(nc, n_cores=NCORES):
    """jit-once mirror of bass2jax.run_bass_via_pjrt's multi-core branch so
    repeated kernel() calls reuse the compiled NEFF."""
    import jax
    import concourse.mybir as mybir
    from concourse.bass2jax import (
        Mesh,
        PartitionSpec,
        _bass_exec_p,
        install_neuronx_cc_hook,
        partition_id_tensor,
        shard_map,
    )

    install_neuronx_cc_hook()
    partition_name = nc.partition_id_tensor.name if nc.partition_id_tensor else None

    in_names, out_names, out_avals, zero_outs = [], [], [], []
    for alloc in nc.m.functions[0].allocations:
        if not isinstance(alloc, mybir.MemoryLocationSet):
            continue
        name = alloc.memorylocations[0].name
        if alloc.kind == "ExternalInput":
            if name != partition_name:
                in_names.append(name)
        elif alloc.kind == "ExternalOutput":
            shape = tuple(alloc.tensor_shape)
            dtype = mybir.dt.np(alloc.dtype)
            out_names.append(name)
            out_avals.append(jax.core.ShapedArray(shape, dtype))
            zero_outs.append(np.zeros(shape, dtype))
    n_params = len(in_names)
    n_outs = len(out_avals)
    all_names = in_names + out_names
    if partition_name is not None:
        all_names = all_names + [partition_name]
    donate = tuple(range(n_params, n_params + n_outs))

    def _body(*args):
        operands = list(args)
        if partition_name is not None:
            operands.append(partition_id_tensor())
        outs = _bass_exec_p.bind(
            *operands,
            out_avals=tuple(out_avals),
            in_names=tuple(all_names),
            out_names=tuple(out_names),
            lowering_input_output_aliases=(),
            sim_require_finite=True,
            sim_require_nnan=True,
            nc=nc,
        )
        return tuple(outs)

    devices = jax.devices()[:n_cores]
    mesh = Mesh(np.asarray(devices), ("core",))
    sharded = jax.jit(
        shard_map(
            _body,
            mesh=mesh,
            in_specs=(PartitionSpec("core"),) * (n_params + n_outs),
            out_specs=(PartitionSpec("core"),) * n_outs,
            check_rep=False,
        ),
        donate_argnums=donate,
        keep_unused=True,
    )

    def prep(in_maps):
        concat_in = [
            np.concatenate([np.asarray(m[name]) for m in in_maps], axis=0)
            for name in in_names
        ]
        return [jax.device_put(a) for a in concat_in]

    def run_dev(dev_in):
        concat_zeros = [
            np.zeros((n_cores * z.shape[0], *z.shape[1:]), z.dtype)
            for z in zero_outs
        ]
        out_arrs = sharded(*dev_in, *concat_zeros)
        out_arrs = [np.asarray(a) for a in out_arrs]
        return [
            {
                name: out_arrs[i].reshape(n_cores, *out_avals[i].shape)[c]
                for i, name in enumerate(out_names)
            }
            for c in range(n_cores)
        ]

    def run(in_maps):
        return run_dev(prep(in_maps))

    run.prep = prep
    run.run_dev = run_dev
    return run


_RUNNER = None


def kernel(P, d_hw, circuit_edge_pairs, circuit_edge_weights, _want_results=False):
    global _RUNNER
    in_maps = _shard_inputs(P, d_hw, circuit_edge_pairs, circuit_edge_weights)
    try:
        if _RUNNER is None:
            _RUNNER = make_runner(_get_built())
        results = _RUNNER(in_maps)
        res = None
    except Exception:
        if _want_results:
            raise
        # fallback: the stock SPMD runner (recompiles per call, but robust)
        from concourse.bass_utils import run_bass_kernel_spmd

        res = run_bass_kernel_spmd(
            _get_built(), in_maps, core_ids=list(range(NCORES))
        )
        results = res.results
    out = _combine(results)
    if _want_results:
        return out, res
    return out



# revision 3
# speedup vs baseline: 1.5718x; 1.5718x over previous
"""Trainium2 Bass kernel for AdjacencyMatchingLoss (8-core SPMD).

Math: adj_score[b,e] = P[b,i_e,:] @ A @ P[b,j_e,:]  with A = (d_hw==1).
Let W[i,j] = sum_e w_e * 1[i_e=i] * 1[j_e=j]   (weighted pair histogram)
and Gm = sum_b P_b A P_b^T scaled by -1/8 (sign + batch mean folded into
the A mask). Then the per-core partial numerator is <W, Gm>.

Structure (v2 — "Gm-first + host one-hot streaming"):
- Host ships PT (P transposed to [q, b*l], bf16). With A in natural [q,r]
  layout, Z_b = matmul(lhsT=Asc, rhs=PT_b) = (P_b A)^T and
  G_b = matmul(lhsT=Z_b, rhs=PT_b) = P_b A P_b^T — no on-device
  transposes. Gm accumulates over b in one PSUM group while the one-hot
  stream is still in flight.
- Host ships the edge one-hots directly as fp8e4m3 ([e-chunk layout,
  128-wide rows]): OhIW carries w * onehot(i), OhJ carries onehot(j)
  (exact 0/1 in fp8). One fused tensor, piece-interleaved
  [IW_p | J_p | ...] so each DMA piece delivers both matmul operands for
  a run of chunks; the PE consumes pieces as they land using DoubleRow
  fp8 matmuls (K=256: two 128-edge chunks per instruction, 0.5
  cycles/row).
- Tail: <W, Gm> = one DVE multiply (W in PSUM x Gm in SBUF) + one
  reduce, then a [128,2] partials DMA ([numerator partial, sum(w)]);
  the host sums partials over partitions/cores and divides (that
  reduction is part of unsharding the scalar output).

The w values ride inside OhIW in fp8 (~2% per-edge rounding, random
sign, averages out over 50k edges: final rel err ~1e-4). P in bf16.

This replaced a DVE-built one-hot design (21us of DVE TensorTensor at
1x — broadcast operands disqualify the 2x/4x DVE modes). CoreSim for
this version predicts ~7us vs 29.5us for the old one.
"""

import os
import sys

import numpy as np

for _p in ("/opt/trn_rl_repo",):
    if os.path.isdir(_p) and _p not in sys.path:
        sys.path.insert(0, _p)

B, NL, NQ, E = 8, 128, 128, 50000
NCORES = 8
ESH = E // NCORES            # 6250 edges per core
CHUNKS = (ESH + 127) // 128  # 49
EPAD = CHUNKS * 128          # 6272

# one-hot stream pieces, in chunk-PAIR units (DoubleRow consumes pairs);
# last piece has the odd single chunk appended
PIECE_CHUNKS = [(0, 14), (14, 26), (26, 38), (38, 49)]

# pm_in packs PT + meta into ONE i16 tensor [128, PM_W] (single DMA):
#   [0:1024)     PT bf16: PT[q, b*128+l] = P[b,l,q]
#   [1024:1073)  w bf16 (per chunk)           -> wsum reduce
#   [1073:1137)  d_hw row as int8 (128 bytes) -> Asc mask
PM_W = 1137

HOST_IW = True  # False: build OhIW on-device via DVE tensor_scalar (4x mode)

_BUILT = None


def _emit_body(nc, sp, pp, tensors):
    import concourse.mybir as mybir

    f32 = mybir.dt.float32
    bf16 = mybir.dt.bfloat16
    i16 = mybir.dt.int16
    i8 = mybir.dt.int8
    fp8 = mybir.dt.float8e4
    EQ = mybir.AluOpType.is_equal
    MUL = mybir.AluOpType.mult
    ADD = mybir.AluOpType.add
    DR = mybir.MatmulPerfMode.DoubleRow
    pt_d, meta_d, oh_d, o_d = tensors

    PT = sp.tile([128, B * NL], bf16)
    meta = sp.tile([128, META_W], i16)
    oh = sp.tile([128, 2 * EPAD], fp8)
    if not HOST_IW:
        OhIW = sp.tile([128, EPAD], bf16)
    Asc = sp.tile([128, NQ], bf16)
    Zsb = sp.tile([128, B * NL], bf16)
    GmS = sp.tile([128, NL], bf16)
    scr = sp.tile([128, NL], f32)
    prt = sp.tile([128, 2], f32)

    Zps = pp.tile([128, B * NL], f32)
    Gps = pp.tile([128, NL], f32)
    Wps = pp.tile([128, NL], f32)

    # ---- DMAs (emitted up front so transfers stream back-to-back) ----
    nc.sync.dma_start(out=meta[:], in_=meta_d.ap())
    nc.sync.dma_start(out=PT[:], in_=pt_d.ap())
    piece_off = []
    off = 0
    for c0, c1 in PIECE_CHUNKS:
        sz = (c1 - c0) * 128
        piece_off.append(off)
        nc.sync.dma_start(
            out=oh[:, off : off + 2 * sz], in_=oh_d.ap()[:, off : off + 2 * sz]
        )
        off += 2 * sz

    def iw_ap(c):
        for (c0, c1), po in zip(PIECE_CHUNKS, piece_off):
            if c0 <= c < c1:
                return po + (c - c0) * 128
        raise AssertionError(c)

    def j_ap(c):
        for (c0, c1), po in zip(PIECE_CHUNKS, piece_off):
            if c0 <= c < c1:
                return po + (c1 - c0) * 128 + (c - c0) * 128
        raise AssertionError(c)

    # views into meta
    iot = meta[:, 0:128].bitcast(bf16)
    idxI = meta[:, 128:177].bitcast(bf16)
    wT = meta[:, 177:226].bitcast(bf16)
    dsb = meta[:, 226:290].bitcast(i8)  # [128, 128] int8

    # ---- small prep ----
    # Asc = -(1/8) * (d_hw == 1): folds sign + batch-mean into the mask
    nc.gpsimd.tensor_scalar(
        out=Asc[:], in0=dsb, scalar1=1, scalar2=-0.125, op0=EQ, op1=MUL
    )
    nc.vector.tensor_reduce(
        out=prt[:, 1:2], in_=wT, axis=mybir.AxisListType.X, op=ADD
    )

    # ---- Gm = sum_b P_b Asc P_b^T via PT-only matmuls ----
    for b in range(B):
        sl = slice(b * 128, (b + 1) * 128)
        nc.tensor.matmul(
            Zps[:, sl], lhsT=Asc[:], rhs=PT[:, sl], start=True, stop=True
        )
    nc.vector.tensor_copy(out=Zsb[:, 0:512], in_=Zps[:, 0:512])
    nc.scalar.copy(out=Zsb[:, 512:1024], in_=Zps[:, 512:1024])
    for b in range(B):
        sl = slice(b * 128, (b + 1) * 128)
        nc.tensor.matmul(
            Gps[:], lhsT=Zsb[:, sl], rhs=PT[:, sl],
            start=(b == 0), stop=(b == B - 1),
        )
    nc.scalar.copy(out=GmS[:], in_=Gps[:])

    # ---- W accumulation from the one-hot stream ----
    if not HOST_IW:
        for c in range(CHUNKS):
            sl = slice(c * 128, (c + 1) * 128)
            nc.vector.tensor_scalar(
                out=OhIW[:, sl], in0=iot, scalar1=idxI[:, c : c + 1],
                scalar2=wT[:, c : c + 1], op0=EQ, op1=MUL,
            )
        for c in range(CHUNKS):
            sl = slice(c * 128, (c + 1) * 128)
            nc.tensor.matmul(
                Wps[:], lhsT=OhIW[:, sl],
                rhs=oh[:, j_ap(c) : j_ap(c) + 128],
                start=(c == 0), stop=(c == CHUNKS - 1),
            )
    else:
        c = 0
        while c < CHUNKS:
            if c + 1 < CHUNKS and iw_ap(c + 1) == iw_ap(c) + 128:
                two = lambda a: oh[:, a : a + 256].rearrange(
                    "p (two m) -> p two m", two=2
                )
                nc.tensor.matmul(
                    Wps[:], lhsT=two(iw_ap(c)), rhs=two(j_ap(c)),
                    start=(c == 0), stop=(c + 2 >= CHUNKS),
                    perf_mode=DR,
                )
                c += 2
            else:
                nc.tensor.matmul(
                    Wps[:], lhsT=oh[:, iw_ap(c) : iw_ap(c) + 128],
                    rhs=oh[:, j_ap(c) : j_ap(c) + 128],
                    start=(c == 0), stop=(c + 1 >= CHUNKS),
                )
                c += 1

    # ---- tail: partial = sum_j W[p,j] * Gm[p,j] ----
    nc.vector.tensor_tensor(out=scr[:], in0=Wps[:], in1=GmS[:], op=MUL)
    nc.vector.tensor_reduce(
        out=prt[:, 0:1], in_=scr[:], axis=mybir.AxisListType.X, op=ADD
    )
    nc.sync.dma_start(out=o_d.ap(), in_=prt[:])


def _build(reps=1):
    import concourse.bacc as bacc
    import concourse.mybir as mybir
    import concourse.tile as tile

    f32 = mybir.dt.float32
    bf16 = mybir.dt.bfloat16
    i16 = mybir.dt.int16
    fp8 = mybir.dt.float8e4

    nc = bacc.Bacc("TRN2", target_bir_lowering=False, debug=False, num_devices=NCORES)

    pt_d = nc.dram_tensor("pt_in", [128, B * NL], bf16, kind="ExternalInput")
    meta_d = nc.dram_tensor("meta_in", [128, META_W], i16, kind="ExternalInput")
    oh_d = nc.dram_tensor("oh_in", [128, 2 * EPAD], fp8, kind="ExternalInput")
    o_d = nc.dram_tensor("out", [128, 2], f32, kind="ExternalOutput")

    with tile.TileContext(nc) as tc:
        with (
            tc.tile_pool(name="sbuf", bufs=1) as sp,
            tc.tile_pool(name="psum", bufs=1, space="PSUM") as pp,
        ):
            for _ in range(reps):
                _emit_body(nc, sp, pp, (pt_d, meta_d, oh_d, o_d))

    nc.compile()
    return nc


def _get_built():
    global _BUILT
    if _BUILT is None:
        _BUILT = _build()
    return _BUILT


def _shard_inputs(P, d_hw, circuit_edge_pairs, circuit_edge_weights):
    import ml_dtypes

    bf16 = ml_dtypes.bfloat16
    fp8 = ml_dtypes.float8_e4m3

    P = np.asarray(P, dtype=np.float32)
    d = np.asarray(d_hw, dtype=np.int32)
    pairs = np.asarray(circuit_edge_pairs).astype(np.int64, copy=False)
    w = np.asarray(circuit_edge_weights, dtype=np.float32)

    # PT[q, b*128 + l] = P[b, l, q]  (replicated to all cores)
    PT = np.ascontiguousarray(
        P.transpose(2, 0, 1).reshape(128, B * NL)
    ).astype(bf16)

    pairs_pad = np.zeros((NCORES, EPAD, 2), dtype=np.int64)
    w_pad = np.zeros((NCORES, EPAD), dtype=np.float32)
    pairs_pad[:, :ESH] = pairs.reshape(NCORES, ESH, 2)
    w_pad[:, :ESH] = w.reshape(NCORES, ESH)

    # edge k (per core) -> chunk cc = k//128, partition p = k%128
    k = np.arange(EPAD)
    cc = k // 128
    p = k % 128
    i_idx = pairs_pad[:, :, 0].astype(np.int64)
    j_idx = pairs_pad[:, :, 1].astype(np.int64)
    core = np.repeat(np.arange(NCORES), EPAD).reshape(NCORES, EPAD)
    pp_b = np.broadcast_to(p, (NCORES, EPAD))

    ohiw_full = np.zeros((NCORES, 128, EPAD), dtype=fp8)
    ohj_full = np.zeros((NCORES, 128, EPAD), dtype=fp8)
    ohiw_full[core, pp_b, cc * 128 + i_idx] = w_pad.astype(fp8)
    ohj_full[core, pp_b, cc * 128 + j_idx] = fp8(1.0)

    # fuse into piece-interleaved layout [IW_piece | J_piece | ...]
    oh = np.zeros((NCORES, 128, 2 * EPAD), dtype=fp8)
    off = 0
    for c0, c1 in PIECE_CHUNKS:
        sz = (c1 - c0) * 128
        oh[:, :, off : off + sz] = ohiw_full[:, :, c0 * 128 : c1 * 128]
        oh[:, :, off + sz : off + 2 * sz] = ohj_full[:, :, c0 * 128 : c1 * 128]
        off += 2 * sz

    # meta: iota | idxI | w | d_hw(int8)
    iot = np.broadcast_to(
        np.arange(128, dtype=bf16), (128, 128)
    )
    idxI_b = (
        i_idx.reshape(NCORES, CHUNKS, 128).transpose(0, 2, 1).astype(bf16)
    )
    w_b = w_pad.reshape(NCORES, CHUNKS, 128).transpose(0, 2, 1).astype(bf16)
    d8 = d.astype(np.int8)  # values 0..3 fit

    meta = np.zeros((NCORES, 128, META_W), dtype=np.int16)
    meta[:, :, 0:128] = np.ascontiguousarray(iot).view(np.int16)[None]
    meta[:, :, 128:177] = np.ascontiguousarray(idxI_b).view(np.int16)
    meta[:, :, 177:226] = np.ascontiguousarray(w_b).view(np.int16)
    meta[:, :, 226:290] = d8.view(np.int16)[None]

    return [
        {
            "pt_in": PT,
            "meta_in": np.ascontiguousarray(meta[i]),
            "oh_in": np.ascontiguousarray(oh[i]),
        }
        for i in range(NCORES)
    ]


def _combine(results):
    parts = np.stack([np.asarray(results[i]["out"]) for i in range(NCORES)])
    numer = float(parts[:, :, 0].astype(np.float64).sum())
    wsum = float(parts[:, :, 1].astype(np.float64).sum())
    return np.float32(numer / max(wsum, 1e-8))


def make_runner(nc, n_cores=NCORES):
    """jit-once mirror of bass2jax.run_bass_via_pjrt's multi-core branch so
    repeated kernel() calls reuse the compiled NEFF."""
    import jax
    import concourse.mybir as mybir
    from concourse.bass2jax import (
        Mesh,
        PartitionSpec,
        _bass_exec_p,
        install_neuronx_cc_hook,
        partition_id_tensor,
        shard_map,
    )

    install_neuronx_cc_hook()
    partition_name = nc.partition_id_tensor.name if nc.partition_id_tensor else None

    in_names, out_names, out_avals, zero_outs = [], [], [], []
    for alloc in nc.m.functions[0].allocations:
        if not isinstance(alloc, mybir.MemoryLocationSet):
            continue
        name = alloc.memorylocations[0].name
        if alloc.kind == "ExternalInput":
            if name != partition_name:
                in_names.append(name)
        elif alloc.kind == "ExternalOutput":
            shape = tuple(alloc.tensor_shape)
            dtype = mybir.dt.np(alloc.dtype)
            out_names.append(name)
            out_avals.append(jax.core.ShapedArray(shape, dtype))
            zero_outs.append(np.zeros(shape, dtype))
    n_params = len(in_names)
    n_outs = len(out_avals)
    all_names = in_names + out_names
    if partition_name is not None:
        all_names = all_names + [partition_name]
    donate = tuple(range(n_params, n_params + n_outs))

    def _body(*args):
        operands = list(args)
        if partition_name is not None:
            operands.append(partition_id_tensor())
        outs = _bass_exec_p.bind(
            *operands,
            out_avals=tuple(out_avals),
            in_names=tuple(all_names),
            out_names=tuple(out_names),
            lowering_input_output_aliases=(),
            sim_require_finite=True,
            sim_require_nnan=True,
            nc=nc,
        )
        return tuple(outs)

    devices = jax.devices()[:n_cores]
    mesh = Mesh(np.asarray(devices), ("core",))
    sharded = jax.jit(
        shard_map(
            _body,
            mesh=mesh,
            in_specs=(PartitionSpec("core"),) * (n_params + n_outs),
            out_specs=(PartitionSpec("core"),) * n_outs,
            check_rep=False,
        ),
        donate_argnums=donate,
        keep_unused=True,
    )

    def prep(in_maps):
        concat_in = [
            np.concatenate([np.asarray(m[name]) for m in in_maps], axis=0)
            for name in in_names
        ]
        return [jax.device_put(a) for a in concat_in]

    def run_dev(dev_in):
        concat_zeros = [
            np.zeros((n_cores * z.shape[0], *z.shape[1:]), z.dtype)
            for z in zero_outs
        ]
        out_arrs = sharded(*dev_in, *concat_zeros)
        out_arrs = [np.asarray(a) for a in out_arrs]
        return [
            {
                name: out_arrs[i].reshape(n_cores, *out_avals[i].shape)[c]
                for i, name in enumerate(out_names)
            }
            for c in range(n_cores)
        ]

    def run(in_maps):
        return run_dev(prep(in_maps))

    run.prep = prep
    run.run_dev = run_dev
    return run


_RUNNER = None


def kernel(P, d_hw, circuit_edge_pairs, circuit_edge_weights, _want_results=False):
    global _RUNNER
    in_maps = _shard_inputs(P, d_hw, circuit_edge_pairs, circuit_edge_weights)
    try:
        if _RUNNER is None:
            _RUNNER = make_runner(_get_built())
        results = _RUNNER(in_maps)
        res = None
    except Exception:
        if _want_results:
            raise
        # fallback: the stock SPMD runner (recompiles per call, but robust)
        from concourse.bass_utils import run_bass_kernel_spmd

        res = run_bass_kernel_spmd(
            _get_built(), in_maps, core_ids=list(range(NCORES))
        )
        results = res.results
    out = _combine(results)
    if _want_results:
        return out, res
    return out
